# revision 1
# baseline (speedup 1.0000x reference)
"""Trainium2 Bass kernel for a 2-layer k-bit-quantized LoRA decoder + quantized lm_head.

Strategy (8 NeuronCores, SPMD):
  - Tensor-parallel, column-parallel everywhere: every quantized weight is
    sharded along its output dim N (q/o/down: 128 rows/core; gate/up: 352;
    k/v: one GQA kv-head (64 rows) replicated per core pair; lm_head: 4000
    vocab rows/core padded to 4096). AllGather (bf16, DRAM bounce) after
    ctx/o/mlp-mid/down re-replicates activations.
  - All activations live transposed on chip: [feature partitions, seq free],
    so matmuls are out[n,s] = w[k,n].T-free / lhsT=w chunk [128k, n<=128],
    rhs = xT [128k, 512s], PSUM accumulates over k-chunks; LoRA (B@(A@x))
    accumulates into the same PSUM bank.
  - Dequant on device: idx staged as uint8 [K, Nsh] (host-transposed),
    ScalarE computes codebook-affine (a*idx + c) -> bf16, the per-(n, block)
    absmax is expanded with a tiny K=2 "selector" matmul into PSUM and
    applied with one VectorE multiply. Codebook must be affine in the index
    (reference uses linspace(-1, 1, 16)); verified at runtime.
  - RMSNorm in transposed layout via ones-column reduce-matmul (sum over
    partitions) and a K=1 broadcast-matmul (which also folds in the norm
    weights); rope via partition-shifted SBUF DMA; causal softmax with a
    staged additive mask, Exp activation with fused accumulate for the
    denominator.
"""

import os
import sys

for _p in ("/opt/trn_rl_repo", "/root/.axon_site/_ro/trn_rl_repo"):
    if os.path.isdir(_p) and _p not in sys.path:
        sys.path.insert(0, _p)

import numpy as np
import ml_dtypes

import concourse.bacc as bacc
import concourse.bass as bass
import concourse.mybir as mybir
import concourse.tile as tile
from concourse import bass_utils

bf16 = ml_dtypes.bfloat16
FP = mybir.dt.float32
BF = mybir.dt.bfloat16
U8 = mybir.dt.uint8
I32 = mybir.dt.int32

NCORES = 8
L = 2
H = 1024
NH = 16
HD = 64
NKV = 4
KVD = NKV * HD
I = 2816
V = 32000
R = 64
S = 512
BLK = 64
NCODE = 16
LORA_S = 16.0 / 64.0
EPS = 1e-6
THETA = 10000.0

HC = H // 128            # 8 hidden chunks
IC = I // 128            # 22 intermediate chunks
ST = S // 128            # 4 seq tiles
N_Q = 128                # q rows per core (2 heads)
N_KV = 64                # kv rows per core (1 gqa head)
N_GU = I // NCORES       # 352
N_D = 128
N_LM = 4096              # padded lm rows per core (4000 real)
LM_REAL = V // NCORES    # 4000
NEG = -1.0e30
ISQ = 1.0 / np.sqrt(HD)

# (idx_key, am_key, A_key, B_key, K_in, N_shard)
PROJS = {
    'q': ('q_idx', 'q_am', 'qA', 'qB', H, N_Q),
    'k': ('k_idx', 'k_am', 'kA', 'kB', H, N_KV),
    'v': ('v_idx', 'v_am', 'vA', 'vB', H, N_KV),
    'o': ('o_idx', 'o_am', 'oA', 'oB', H, N_Q),
    'g': ('g_idx', 'g_am', 'gA', 'gB', H, N_GU),
    'u': ('u_idx', 'u_am', 'uA', 'uB', H, N_GU),
    'd': ('d_idx', 'd_am', 'dA', 'dB', I, N_D),
}


def _shard_rows(p, r):
    if p in ('q', 'o', 'd'):
        return slice(128 * r, 128 * (r + 1))
    if p in ('k', 'v'):
        kvh = r // 2
        return slice(64 * kvh, 64 * (kvh + 1))
    if p in ('g', 'u'):
        return slice(N_GU * r, N_GU * (r + 1))
    raise KeyError(p)


def _rope_tables():
    inv_freq = 1.0 / (THETA ** (np.arange(0, HD, 2, dtype=np.float32) / HD))
    freqs = np.outer(np.arange(S, dtype=np.float32), inv_freq)
    emb = np.concatenate([freqs, freqs], axis=-1)          # [S, HD]
    cosT = np.cos(emb).T.astype(np.float32)                # [HD, S]
    sinT = np.sin(emb).T.astype(np.float32)
    sinT[:HD // 2] *= -1.0                                 # sign for rotate_half
    cos_rep = np.tile(cosT, (2, 1)).astype(bf16)           # [128, S]
    sin_rep = np.tile(sinT, (2, 1)).astype(bf16)
    return cos_rep, sin_rep


def _mask_table():
    m = np.zeros((128, 128), dtype=bf16)
    for i in range(128):
        m[i, i + 1:] = NEG
    return m


def _amT(am_flat, rows, n_out, k_in):
    """[KB, Nsh] bf16: transposed per-block absmax for the row shard."""
    kb = k_in // BLK
    am_mat = np.asarray(am_flat, np.float32).reshape(n_out, kb)[rows]  # [Nsh, kb]
    return np.ascontiguousarray(am_mat.T).astype(bf16)


def _tsel(k_in):
    """[KB, KC*128] bf16 selector: T[b, c*128+p] = 1 iff b == 2c + p//64."""
    kb = k_in // BLK
    kc = k_in // 128
    t = np.zeros((kb, kc * 128), dtype=bf16)
    for c in range(kc):
        t[2 * c, c * 128:c * 128 + 64] = 1
        t[2 * c + 1, c * 128 + 64:(c + 1) * 128] = 1
    return t


def _build_in_maps(inputs):
    """Per-core input dicts (host sharding/layout only)."""
    maps = []
    embed = np.ascontiguousarray(np.asarray(inputs['embed'], np.float32))
    ids = np.ascontiguousarray(np.asarray(inputs['input_ids'], np.int32)).reshape(1, S)
    lm_idxT_full = np.asarray(inputs['lm_idx'], np.int64)
    lm_am = np.asarray(inputs['lm_am'], np.float32)
    for r in range(NCORES):
        m = {'ids': ids, 'embed': embed}
        for l in range(L):
            for p, (ik, ak, Ak, Bk, K, Nsh) in PROJS.items():
                rows = _shard_rows(p, r)
                idx = np.asarray(inputs[ik][l])
                m[f'idx_{p}{l}'] = np.ascontiguousarray(idx[rows].T).astype(np.uint8)
                m[f'am_{p}{l}'] = _amT(inputs[ak][l], rows, idx.shape[0], K)
                m[f'apt_{p}{l}'] = np.ascontiguousarray(
                    (LORA_S * np.asarray(inputs[Ak][l], np.float32)).T).astype(bf16)
                m[f'bt_{p}{l}'] = np.ascontiguousarray(
                    np.asarray(inputs[Bk][l], np.float32)[rows].T).astype(bf16)
            m[f'ln1_{l}'] = np.ascontiguousarray(
                np.asarray(inputs['ln1'][l], np.float32).reshape(1, H)).astype(bf16)
            m[f'ln2_{l}'] = np.ascontiguousarray(
                np.asarray(inputs['ln2'][l], np.float32).reshape(1, H)).astype(bf16)
        m['fnorm'] = np.ascontiguousarray(
            np.asarray(inputs['final_norm'], np.float32).reshape(1, H)).astype(bf16)
        # lm head shard: rows [4000r, 4000(r+1)) padded to 4096
        lo = LM_REAL * r
        sl = lm_idxT_full[lo:lo + LM_REAL]                      # [4000, 1024]
        idxp = np.zeros((N_LM, H), dtype=np.uint8)
        idxp[:LM_REAL] = sl
        m['idx_lm'] = np.ascontiguousarray(idxp.T).astype(np.uint8)   # [1024, 4096]
        amp_ = np.zeros((N_LM, H // BLK), dtype=np.float32)
        amp_[:LM_REAL] = lm_am.reshape(V, H // BLK)[lo:lo + LM_REAL]
        m['am_lm'] = np.ascontiguousarray(amp_.T).astype(bf16)   # [16, 4096]
        maps.append(m)
    return maps


def _build_program(a_cb, c_cb):
    nc = bacc.Bacc("TRN2", target_bir_lowering=False, debug=False,
                   enable_asserts=False, num_devices=NCORES)

    # --- dram I/O ----------------------------------------------------------
    d_ids = nc.dram_tensor('ids', [1, S], I32, kind="ExternalInput")
    d_embed = nc.dram_tensor('embed', [V, H], FP, kind="ExternalInput")
    d = {}
    for l in range(L):
        for p, (ik, ak, Ak, Bk, K, Nsh) in PROJS.items():
            kc = K // 128
            d[f'idx_{p}{l}'] = nc.dram_tensor(f'idx_{p}{l}', [K, Nsh], U8, kind="ExternalInput")
            d[f'am_{p}{l}'] = nc.dram_tensor(f'am_{p}{l}', [K // BLK, Nsh], BF, kind="ExternalInput")
            d[f'apt_{p}{l}'] = nc.dram_tensor(f'apt_{p}{l}', [K, R], BF, kind="ExternalInput")
            d[f'bt_{p}{l}'] = nc.dram_tensor(f'bt_{p}{l}', [R, Nsh], BF, kind="ExternalInput")
        d[f'ln1_{l}'] = nc.dram_tensor(f'ln1_{l}', [1, H], BF, kind="ExternalInput")
        d[f'ln2_{l}'] = nc.dram_tensor(f'ln2_{l}', [1, H], BF, kind="ExternalInput")
    d['fnorm'] = nc.dram_tensor('fnorm', [1, H], BF, kind="ExternalInput")
    d['idx_lm'] = nc.dram_tensor('idx_lm', [H, N_LM], U8, kind="ExternalInput")
    d['am_lm'] = nc.dram_tensor('am_lm', [H // BLK, N_LM], BF, kind="ExternalInput")
    d_out = nc.dram_tensor('out', [N_LM, S], FP, kind="ExternalOutput")

    # --- NEFF-inline constants --------------------------------------------
    c_sel16 = nc.inline_tensor(_tsel(H), 'c_sel16')     # [16, 1024]
    c_sel44 = nc.inline_tensor(_tsel(I), 'c_sel44')     # [44, 2816]
    c_identb = nc.inline_tensor(np.eye(128, dtype=bf16), 'c_identb')
    c_identf = nc.inline_tensor(np.eye(128, dtype=np.float32), 'c_identf')
    c_onescol = nc.inline_tensor(np.ones((128, 1), dtype=bf16), 'c_onescol')
    cos_rep, sin_rep = _rope_tables()
    c_cos = nc.inline_tensor(cos_rep, 'c_cos')
    c_sin = nc.inline_tensor(sin_rep, 'c_sin')
    c_mask = nc.inline_tensor(_mask_table(), 'c_mask')  # [128,128] bf16 triangle

    with tile.TileContext(nc) as tc:
        ctxs = []
        def pool(**kw):
            p = tc.tile_pool(**kw)
            ctxs.append(p)
            return p.__enter__()

        cpool = pool(name="const", bufs=1)
        hpool = pool(name="h", bufs=1)
        epool = pool(name="e", bufs=2)        # embed gather tiles
        xpool = pool(name="x", bufs=HC)
        wpool = pool(name="w", bufs=6)
        lmwpool = pool(name="lmw", bufs=10)   # lm-head weight tiles (8 live + prefetch)
        spool = pool(name="s", bufs=3)        # misc working tiles
        zpool = pool(name="z", bufs=2)
        fpool = pool(name="f", bufs=8)        # allgathered full activations
        dram = pool(name="dram", bufs=1, space="DRAM")
        psA = pool(name="psA", bufs=3, space="PSUM")   # am expansion / transposes / bcast
        psY = pool(name="psY", bufs=3, space="PSUM")   # matmul outputs / scores
        psZ = pool(name="psZ", bufs=2, space="PSUM")   # lora z / ctx / rms reduce

        # constants to SBUF
        SEL16 = cpool.tile([16, HC * 128], BF, tag="SEL16")
        nc.sync.dma_start(SEL16[:], c_sel16.ap())
        SEL44 = cpool.tile([I // BLK, IC * 128], BF, tag="SEL44")
        nc.sync.dma_start(SEL44[:], c_sel44.ap())
        IDB = cpool.tile([128, 128], BF, tag="IDB")
        nc.sync.dma_start(IDB[:], c_identb.ap())
        IDF = cpool.tile([128, 128], FP, tag="IDF")
        nc.sync.dma_start(IDF[:], c_identf.ap())
        ONESC = cpool.tile([128, 1], BF, tag="ONESC")
        nc.sync.dma_start(ONESC[:], c_onescol.ap())
        COS = cpool.tile([128, S], BF, tag="COS")
        nc.sync.dma_start(COS[:], c_cos.ap())
        SIN = cpool.tile([128, S], BF, tag="SIN")
        nc.sync.dma_start(SIN[:], c_sin.ap())
        MASK = cpool.tile([128, 128], BF, tag="MASK")
        nc.sync.dma_start(MASK[:], c_mask.ap())
        LNW = {}
        for l in range(L):
            for nm in (f'ln1_{l}', f'ln2_{l}'):
                t = cpool.tile([1, H], BF, tag=nm)
                nc.sync.dma_start(t[:], d[nm].ap())
                LNW[nm] = t
        t = cpool.tile([1, H], BF, tag='fnorm')
        nc.sync.dma_start(t[:], d['fnorm'].ap())
        LNW['fnorm'] = t
        epst = cpool.tile([1, 1], FP, tag='epst')
        nc.vector.memset(epst[:], EPS)

        # --- embedding gather + transpose to hT (f32) ---------------------
        idst = spool.tile([128, ST], I32, tag="idst")
        nc.sync.dma_start(idst[:], d_ids.ap()[0, :].rearrange("(t p) -> p t", p=128))
        hT = []
        for c in range(HC):
            ht = hpool.tile([128, S], FP, tag=f"h{c}")
            hT.append(ht)
        for t in range(ST):
            h0 = epool.tile([128, H], FP, tag="h0")
            nc.gpsimd.indirect_dma_start(
                out=h0[:], out_offset=None, in_=d_embed.ap(),
                in_offset=bass.IndirectOffsetOnAxis(ap=idst[:, t:t + 1], axis=0))
            for c in range(HC):
                ps = psA.tile([128, 128], FP, tag="amp")
                nc.tensor.matmul(ps[:], h0[:, c * 128:(c + 1) * 128], IDF[:],
                                 is_transpose=True, start=True, stop=True)
                nc.scalar.copy(hT[c][:, t * 128:(t + 1) * 128], ps[:])

        # --- helpers -------------------------------------------------------
        def rmsnorm(lnw_tile):
            """hT (f32) -> new xT bf16 list."""
            ssp = psZ.tile([1, S], FP, tag="z")
            for c in range(HC):
                sq = spool.tile([128, S], BF, tag="sq")
                nc.scalar.square(sq[:], hT[c][:])
                nc.tensor.matmul(ssp[:], ONESC[:], sq[:],
                                 start=(c == 0), stop=(c == HC - 1))
            sroot = spool.tile([1, S], FP, tag="sroot")
            nc.scalar.activation(sroot[:], ssp[:], mybir.ActivationFunctionType.Sqrt,
                                 bias=epst[:], scale=1.0 / H)
            rinv = spool.tile([1, S], FP, tag="rinv")
            nc.vector.reciprocal(rinv[:], sroot[:])
            rinvb = spool.tile([1, S], BF, tag="rinvb")
            nc.vector.tensor_copy(rinvb[:], rinv[:])
            xs = []
            for c in range(HC):
                bc = psA.tile([128, S], FP, tag="amp")
                nc.tensor.matmul(bc[:], lnw_tile[:, c * 128:(c + 1) * 128], rinvb[:],
                                 start=True, stop=True)
                xt = xpool.tile([128, S], BF, tag="xT")
                nc.vector.tensor_tensor(xt[:], hT[c][:], bc[:], mybir.AluOpType.mult)
                xs.append(xt)
            return xs

        def dequant(idx_d, am_t, sel, kb, Nsh, c, ncols=None, coloff=0):
            """Dequant k-chunk c (cols [coloff, coloff+ncols)) -> bf16 [128, ncols]."""
            if ncols is None:
                ncols = Nsh
            idxc = spool.tile([128, ncols], U8, tag=f"idx{ncols}")
            nc.sync.dma_start(idxc[:], idx_d.ap()[c * 128:(c + 1) * 128,
                                                  coloff:coloff + ncols])
            cbv = spool.tile([128, ncols], BF, tag=f"cbv{ncols}")
            nc.scalar.activation(cbv[:], idxc[:], mybir.ActivationFunctionType.Copy,
                                 bias=float(c_cb), scale=float(a_cb))
            amp = psA.tile([128, ncols], FP, tag="amp")
            nc.tensor.matmul(amp[:], sel[:kb, c * 128:(c + 1) * 128],
                             am_t[:kb, coloff:coloff + ncols],
                             start=True, stop=True)
            wp = lmwpool if ncols == 512 else wpool
            wt = wp.tile([128, ncols], BF, tag=f"w{ncols}")
            nc.vector.tensor_tensor(wt[:], cbv[:], amp[:], mybir.AluOpType.mult)
            return wt

        def load_am(p, l):
            K, Nsh = PROJS[p][4], PROJS[p][5]
            t = spool.tile([K // BLK, Nsh], BF, tag=f"am_{p}")
            nc.sync.dma_start(t[:], d[f'am_{p}{l}'].ap())
            return t

        def lora_z(apt_d, K, rhs_chunks, tag="zz"):
            """z = (LORA_S*A) @ x  -> bf16 [64, S]."""
            kc = K // 128
            zp = psZ.tile([R, S], FP, tag="z")
            for c in range(kc):
                ap_t = spool.tile([128, R], BF, tag="aptc")
                nc.sync.dma_start(ap_t[:], apt_d.ap()[c * 128:(c + 1) * 128, :])
                nc.tensor.matmul(zp[:], ap_t[:], rhs_chunks[c][:],
                                 start=(c == 0), stop=(c == kc - 1))
            z = zpool.tile([R, S], BF, tag=tag)
            nc.scalar.copy(z[:], zp[:])
            return z

        def proj(p, l, rhs_chunks, n_tiles):
            """Full quantized+lora projection; returns list of psum tiles [nt]."""
            ik, ak, Ak, Bk, K, Nsh = PROJS[p]
            kc = K // 128
            amt = load_am(p, l)
            z = lora_z(d[f'apt_{p}{l}'], K, rhs_chunks)
            bt = spool.tile([R, Nsh], BF, tag=f"bt_{p}")
            nc.sync.dma_start(bt[:], d[f'bt_{p}{l}'].ap())
            psums = []
            for nt in range(n_tiles):
                n0 = nt * 128
                nw = min(128, Nsh - n0)
                ps = psY.tile([nw, S], FP, tag="y")
                psums.append((ps, nw))
            sel = SEL44 if K == I else SEL16
            for c in range(kc):
                wt = dequant(d[f'idx_{p}{l}'], amt, sel, K // BLK, Nsh, c)
                for nt, (ps, nw) in enumerate(psums):
                    nc.tensor.matmul(ps[:], wt[:, nt * 128:nt * 128 + nw],
                                     rhs_chunks[c][:], start=(c == 0), stop=False)
            for nt, (ps, nw) in enumerate(psums):
                nc.tensor.matmul(ps[:], bt[:, nt * 128:nt * 128 + nw], z[:],
                                 start=False, stop=True)
            return psums

        def allgather(in_tiles, nsh, name):
            """AG bf16 shards [nsh, S] -> full [(8*nsh), S] chunk tiles [128, S]."""
            bin_ = dram.tile([nsh, S], BF, tag=f"agi_{name}")
            off = 0
            for t, rows in in_tiles:
                nc.sync.dma_start(bin_[off:off + rows, :], t[:rows, :])
                off += rows
            bout = dram.tile([NCORES * nsh, S], BF, tag=f"ago_{name}",
                             addr_space="Shared")
            nc.gpsimd.collective_compute(
                "AllGather", mybir.AluOpType.bypass,
                replica_groups=[list(range(NCORES))],
                ins=[bin_.opt()], outs=[bout.opt()])
            chunks = []
            total = NCORES * nsh
            for c in range(total // 128):
                f = fpool.tile([128, S], BF, tag="fc")
                nc.sync.dma_start(f[:], bout[c * 128:(c + 1) * 128, :])
                chunks.append(f)
            return chunks

        def rope(xt, rows, tag):
            """in-place-ish rope on [rows, S] bf16 tile (rows 64 or 128)."""
            sh = spool.tile([rows, S], BF, tag=f"sh_{tag}")
            for b in range(rows // 64):
                p0 = b * 64
                nc.sync.dma_start(sh[p0:p0 + 32, :], xt[p0 + 32:p0 + 64, :])
                nc.sync.dma_start(sh[p0 + 32:p0 + 64, :], xt[p0:p0 + 32, :])
            rot = spool.tile([rows, S], BF, tag=f"rot_{tag}")
            nc.vector.tensor_tensor(rot[:], xt[:], COS[:rows, :], mybir.AluOpType.mult)
            nc.vector.tensor_tensor(sh[:], sh[:], SIN[:rows, :], mybir.AluOpType.mult)
            nc.vector.tensor_add(rot[:], rot[:], sh[:])
            return rot

        # --- layers --------------------------------------------------------
        for l in range(L):
            xs = rmsnorm(LNW[f'ln1_{l}'])
            (qps, _), = proj('q', l, xs, 1)
            (kps, _), = proj('k', l, xs, 1)
            (vps, _), = proj('v', l, xs, 1)
            qT = spool.tile([128, S], BF, tag="qT")
            nc.scalar.copy(qT[:], qps[:])
            kT = spool.tile([64, S], BF, tag="kT")
            nc.scalar.copy(kT[:], kps[:])
            vT = spool.tile([64, S], BF, tag="vT")
            nc.scalar.copy(vT[:], vps[:])
            qR = rope(qT, 128, "q")
            kR = rope(kT, 64, "k")
            # second q head to its own base-0 tile
            qh1 = spool.tile([64, S], BF, tag="qh1")
            nc.sync.dma_start(qh1[:], qR[64:128, :])
            # transpose v -> [S, 64] tiles
            vv = []
            for t in range(ST):
                vp = psA.tile([128, 64], BF, tag="amp")
                nc.tensor.matmul(vp[:], vT[:, t * 128:(t + 1) * 128], IDB[:64, :64],
                                 is_transpose=True, start=True, stop=True)
                vs = spool.tile([128, 64], BF, tag=f"vv{t}")
                nc.scalar.copy(vs[:], vp[:])
                vv.append(vs)
            ctxT = spool.tile([128, S], BF, tag="ctxT")
            for hh in range(2):
                qh = qR if hh == 0 else qh1
                cps = psZ.tile([64, S], FP, tag="z")
                for t in range(ST):
                    W = (t + 1) * 128      # causal: only keys <= (t+1)*128
                    sp = psY.tile([128, W], FP, tag="y")
                    nc.tensor.matmul(sp[:], qh[:64, t * 128:(t + 1) * 128],
                                     kR[:, :W], start=True, stop=True)
                    ssb = spool.tile([128, W], FP, tag="ssb")
                    if t > 0:
                        nc.vector.tensor_copy(ssb[:, :t * 128], sp[:, :t * 128])
                    nc.vector.tensor_add(ssb[:, t * 128:W], sp[:, t * 128:W],
                                         MASK[:])
                    mx = spool.tile([128, 1], FP, tag="mx")
                    nc.vector.tensor_reduce(mx[:], ssb[:], mybir.AxisListType.X,
                                            mybir.AluOpType.max)
                    nmx = spool.tile([128, 1], FP, tag="nmx")
                    nc.vector.tensor_scalar_mul(nmx[:], mx[:], -ISQ)
                    att = spool.tile([128, W], BF, tag="att")
                    sm = spool.tile([128, 1], FP, tag="sm")
                    nc.scalar.activation(att[:], ssb[:],
                                         mybir.ActivationFunctionType.Exp,
                                         bias=nmx[:], scale=ISQ, accum_out=sm[:])
                    rs = spool.tile([128, 1], FP, tag="rs")
                    nc.vector.reciprocal(rs[:], sm[:])
                    nc.vector.tensor_scalar(att[:], att[:], rs[:], None,
                                            mybir.AluOpType.mult)
                    for u in range(t + 1):
                        ap_ = psA.tile([128, 128], BF, tag="amp")
                        nc.tensor.matmul(ap_[:], att[:, u * 128:(u + 1) * 128], IDB[:],
                                         is_transpose=True, start=True, stop=True)
                        asb = spool.tile([128, 128], BF, tag="asb")
                        nc.scalar.copy(asb[:], ap_[:])
                        nc.tensor.matmul(cps[:, t * 128:(t + 1) * 128], vv[u][:],
                                         asb[:], start=(u == 0), stop=(u == t))
                nc.scalar.copy(ctxT[hh * 64:(hh + 1) * 64, :], cps[:])
            ctx_full = allgather([(ctxT, 128)], 128, f"ctx{l}")
            (ops_, _), = proj('o', l, ctx_full, 1)
            oT = spool.tile([128, S], BF, tag="oT")
            nc.scalar.copy(oT[:], ops_[:])
            o_full = allgather([(oT, 128)], 128, f"o{l}")
            for c in range(HC):
                nc.vector.tensor_add(hT[c][:], hT[c][:], o_full[c][:])

            xs2 = rmsnorm(LNW[f'ln2_{l}'])
            gps = proj('g', l, xs2, 3)
            gts = []
            for ps, nw in gps:
                gt = spool.tile([nw, S], BF, tag="gt")
                nc.scalar.activation(gt[:], ps[:], mybir.ActivationFunctionType.Silu)
                gts.append(gt)
            ups = proj('u', l, xs2, 3)
            mts = []
            for (ps, nw), gt in zip(ups, gts):
                mt = spool.tile([nw, S], BF, tag="mt")
                nc.vector.tensor_tensor(mt[:], gt[:], ps[:], mybir.AluOpType.mult)
                mts.append(mt)
            m_full = allgather([(mts[0], 128), (mts[1], 128), (mts[2], 96)],
                               N_GU, f"m{l}")
            (dps, _), = proj('d', l, m_full, 1)
            dT = spool.tile([128, S], BF, tag="dT")
            nc.scalar.copy(dT[:], dps[:])
            d_full = allgather([(dT, 128)], 128, f"d{l}")
            for c in range(HC):
                nc.vector.tensor_add(hT[c][:], hT[c][:], d_full[c][:])

        # --- final norm + lm head -----------------------------------------
        xlm = rmsnorm(LNW['fnorm'])
        am_lm = cpool.tile([H // BLK, N_LM], BF, tag="am_lm")
        nc.sync.dma_start(am_lm[:], d['am_lm'].ap())
        for nb in range(N_LM // 512):
            wts = []
            for c in range(HC):
                wt = dequant(d['idx_lm'], am_lm, SEL16, H // BLK, N_LM, c,
                             ncols=512, coloff=nb * 512)
                wts.append(wt)
            for nt in range(4):
                ps = psY.tile([128, S], FP, tag="y")
                for c in range(HC):
                    nc.tensor.matmul(ps[:], wts[c][:, nt * 128:(nt + 1) * 128],
                                     xlm[c][:], start=(c == 0), stop=(c == HC - 1))
                lo = spool.tile([128, S], FP, tag="lo")
                nc.vector.tensor_copy(lo[:], ps[:])
                nc.sync.dma_start(d_out.ap()[nb * 512 + nt * 128:
                                             nb * 512 + (nt + 1) * 128, :], lo[:])

        for p in reversed(ctxs):
            p.__exit__(None, None, None)
    nc.compile()
    return nc


_prog_cache = {}


def _get_program(a_cb, c_cb):
    key = (round(float(a_cb), 9), round(float(c_cb), 9))
    if key not in _prog_cache:
        _prog_cache[key] = _build_program(a_cb, c_cb)
    return _prog_cache[key]


def kernel(**inputs):
    cb = np.asarray(inputs['codebook'], np.float32)
    idxs = np.arange(NCODE, dtype=np.float32)
    a_cb = float((cb[-1] - cb[0]) / (NCODE - 1))
    c_cb = float(cb[0])
    resid = np.abs(cb - (a_cb * idxs + c_cb)).max()
    if resid > 1e-5 * max(1.0, np.abs(cb).max()):
        # general (non-affine) codebook: refit by least squares; warn loudly.
        A = np.stack([idxs, np.ones_like(idxs)], 1)
        sol, *_ = np.linalg.lstsq(A, cb, rcond=None)
        a_cb, c_cb = float(sol[0]), float(sol[1])
        print(f"WARNING: codebook is not affine (resid={resid:.3e}); "
              f"kernel uses affine fit and may lose accuracy", file=sys.stderr)

    in_maps = _build_in_maps(inputs)
    nc = _get_program(a_cb, c_cb)
    res = bass_utils.run_bass_kernel_spmd(
        nc, in_maps, core_ids=list(range(NCORES)),
        trace=bool(int(os.environ.get('KBIT_TRACE', '0'))))
    outs = [res.results[r]['out'][:LM_REAL] for r in range(NCORES)]
    logits = np.concatenate(outs, axis=0).T.reshape(1, S, V).astype(np.float32)
    kernel.last_results = res
    return logits


def timed_run(inputs, iters=4):
    """Stage inputs once, then time repeated NEFF executions (returns list of
    per-iteration wall seconds around the sharded PJRT call, inputs resident)."""
    import time
    import jax
    from jax.sharding import Mesh, PartitionSpec, NamedSharding
    from jax.experimental.shard_map import shard_map
    from concourse import bass2jax, mybir as _mb

    cb = np.asarray(inputs['codebook'], np.float32)
    a_cb = float((cb[-1] - cb[0]) / (NCODE - 1))
    c_cb = float(cb[0])
    in_maps = _build_in_maps(inputs)
    nc = _get_program(a_cb, c_cb)
    bass2jax.install_neuronx_cc_hook()

    in_names, out_names, out_avals, zero_outs = [], [], [], []
    for alloc in nc.m.functions[0].allocations:
        if not isinstance(alloc, _mb.MemoryLocationSet):
            continue
        name = alloc.memorylocations[0].name
        pname = nc.partition_id_tensor.name if nc.partition_id_tensor else None
        if alloc.kind == "ExternalInput":
            if name != pname:
                in_names.append(name)
        elif alloc.kind == "ExternalOutput":
            out_names.append(name)
            npdt = _mb.dt.np(alloc.dtype)
            out_avals.append(jax.core.ShapedArray(tuple(alloc.tensor_shape), npdt))
            zero_outs.append(np.zeros(tuple(alloc.tensor_shape), npdt))
    n_params = len(in_names)
    n_outs = len(out_names)
    all_in = in_names + out_names

    pname = nc.partition_id_tensor.name if nc.partition_id_tensor else None
    if pname:
        all_in.append(pname)

    def _body(*args):
        ops = list(args)
        if pname:
            ops.append(bass2jax.partition_id_tensor())
        outs = bass2jax._bass_exec_p.bind(
            *ops, out_avals=tuple(out_avals), in_names=tuple(all_in),
            out_names=tuple(out_names), lowering_input_output_aliases=(),
            sim_require_finite=True, sim_require_nnan=True, nc=nc)
        return tuple(outs)

    devices = jax.devices()[:NCORES]
    mesh = Mesh(np.asarray(devices), ("core",))
    in_specs = (PartitionSpec("core"),) * (n_params + n_outs)
    out_specs = (PartitionSpec("core"),) * n_outs
    fn = jax.jit(shard_map(_body, mesh=mesh, in_specs=in_specs,
                           out_specs=out_specs, check_rep=False),
                 keep_unused=True)
    sh = NamedSharding(mesh, PartitionSpec("core"))
    concat_in = [
        jax.device_put(
            np.concatenate([np.asarray(in_maps[c][nm]) for c in range(NCORES)], 0), sh)
        for nm in in_names]
    concat_zeros = [
        jax.device_put(np.zeros((NCORES * z.shape[0], *z.shape[1:]), z.dtype), sh)
        for z in zero_outs]
    for x in concat_in + concat_zeros:
        x.block_until_ready()
    times = []
    out = None
    for it in range(iters):
        t0 = time.perf_counter()
        out = fn(*concat_in, *concat_zeros)
        jax.block_until_ready(out)
        times.append(time.perf_counter() - t0)
    outs = np.asarray(out[0]).reshape(NCORES, *out_avals[0].shape)
    logits = np.concatenate([outs[r][:LM_REAL] for r in range(NCORES)], 0)
    logits = logits.T.reshape(1, S, V).astype(np.float32)
    return times, logits

